# revision 1
# baseline (speedup 1.0000x reference)
"""MoE routed dynamics kernel for Trainium2 (8 NeuronCores, expert-parallel).

Problem: for each row b of a [B, D+A] input, route through one of P=8
two-layer MLPs selected by policy_indices[b]:
    h = relu(x @ W1[p] + b1[p]);  y = h @ W2[p] + b2[p]

Sharding: expert-parallel. Core p owns expert p's weights and processes
the rows routed to expert p. The all-to-all dispatch keyed on
policy_indices happens on the host at shard time (gather rows by expert,
pad to a common capacity C multiple of 128), and the inverse scatter at
unshard time.

Design notes (from iterative trace analysis; ~110us -> ~97us):
- Matmul dtype is float32r end-to-end: measured steady-state pitch for
  an N=512 matmul is ~229ns (fp32r) vs ~259ns (bf16) on this silicon —
  fp32r HIGH mode streams ~13% faster and needs no input quantization.
  fp32r requires N>=256 to stream at 1 cycle/col, so every column chunk
  is >=256 wide. C is the exact max expert count (rounded to 8 cols);
  every padded column costs 72 PE passes.
- The kernel is split into an L1 phase (all chunks) and an L2 phase
  (all chunks, reversed). All h tiles stay resident in SBUF
  (~67KB/partition). This pushes W2's 2MB load out of the startup
  window, which is the bottleneck: all 8 cores load concurrently and
  the per-core DMA bus (~360GB/s, billed at SBUF-write size) saturates.
- Loads are few (one per logical block, ~15 total vs 81 in the old
  baseline whose Sync-engine DMA issues alone serialized 51us) and are
  spread across sync/scalar (HWDGE) and gpsimd (SWDGE, ~25ns dispatch)
  in first-use order. x and w1's later column blocks arrive bf16-staged
  (half the bus bytes) and the idle ACT/DVE engines cast them up to
  float32r, split so neither queue delays chunk0's relus; x2+ transfers
  are released by a gpsimd read of an earlier chunk's h tile
  (in-order-queue flow control) so they never compete with
  startup-critical loads.
- The PE clock ramps for the first ~15-20us of the kernel regardless of
  activity; warmup matmuls on a zeroed tile burn the x0 DMA wait.
- Output is stored bf16 and upcast on the host (error ~0.2%, halves
  store bytes); the final store is a single small d-group so the kernel
  tail is minimal. The remaining fixed overhead is the NEFF prologue
  (~6.7us of barriers/init) and epilogue (~8.5us of semaphore resets),
  plus ~5us of reduced-clock (cold) PE time — none controllable from
  kernel code.
"""

import math

import numpy as np
import ml_dtypes

_B = 16384
_P = 8
_D = 512
_A = 64
_H = 1024
_DA = _D + _A   # 576
_KC = 5         # K chunks over DA padded to 5*128=640
_N_CORES = 8

_kernel_cache: dict = {}


def _chunks(C: int):
    """Column chunking: 256-wide lead-in (small first x transfer, so the
    PE starts sooner), 512-wide steady chunks, and a tail split that
    keeps every chunk >=256 (fp32r needs N>=256 for full rate)."""
    assert C >= 256, C
    if C <= 512:
        return [C]
    out = [256]
    rem = C - 256
    while rem >= 1024:
        out.append(512)
        rem -= 512
    if rem <= 512:
        out.append(rem)
    elif rem - 512 >= 256:
        out += [512, rem - 512]
    else:
        out += [rem - 256, 256]
    return out


def _build_bass(C: int):
    import concourse.bacc as bacc
    import concourse.mybir as mybir
    from concourse.tile import TileContext

    fp32 = mybir.dt.float32
    f32r = mybir.dt.float32r
    bf16 = mybir.dt.bfloat16
    act = mybir.ActivationFunctionType

    widths = _chunks(C)
    offsets = [sum(widths[:i]) for i in range(len(widths))]
    mh = _H // 128  # 8 L1 output groups
    md = _D // 128  # 4 L2 output groups

    nc = bacc.Bacc()
    # x lives in DRAM as bf16 (staged and DVE-upcast); weights are
    # float32r.
    # xq is chunk-major: chunk ci occupies flat cols [KC*n0, KC*(n0+nl))
    # contiguously per partition, so each chunk load is ONE descriptor
    # per partition (vs 5 x 512B segments = 5x the packet overhead).
    xd = nc.declare_dram_parameter("xq", [128, _KC * C], bf16, isOutput=False)
    w1d = nc.declare_dram_parameter("w1q", [128, _KC, 384], f32r, isOutput=False)
    w1bd = nc.declare_dram_parameter("w1qb", [128, _KC * (_H - 384)], bf16, isOutput=False)
    w2d = nc.declare_dram_parameter("w2q", [128, mh, _D], f32r, isOutput=False)
    bd = nc.declare_dram_parameter("bq", [128, mh + md], fp32, isOutput=False)
    od = nc.declare_dram_parameter("oq", [128, md, C], bf16, isOutput=True)

    with TileContext(nc) as tc:
        with (
            tc.tile_pool(name="wpool", bufs=1) as wpool,
            tc.tile_pool(name="xpool", bufs=len(widths)) as xpool,
            tc.tile_pool(name="xbpool", bufs=2) as xbpool,
            tc.tile_pool(name="hpool", bufs=1) as hpool,
            tc.tile_pool(name="ypool", bufs=2) as ypool,
            tc.tile_pool(name="psum", bufs=8, space="PSUM") as psp,
        ):
            w1_sb = wpool.tile([128, _KC, _H], f32r, tag="w1")
            w2_sb = wpool.tile([128, mh, _D], f32r, tag="w2")
            b_sb = wpool.tile([128, mh + md], fp32, tag="b")
            warm_sb = wpool.tile([128, 640], bf16, tag="warm")

            x_sb = [
                xpool.tile([128, _KC, nl], f32r, tag="x", name=f"x{ci}")
                for ci, nl in enumerate(widths)
            ]
            # bf16 staging tiles for x1+: the DMA moves half the bytes
            # (the load phase is DMA-bus-bound) and the idle-during-L1
            # DVE upcasts them to float32r.
            xb_sb = [
                xbpool.tile([128, _KC, nl], bf16, tag="xb", name=f"xb{ci}")
                for ci, nl in enumerate(widths)
            ]

            # The load phase is limited by the per-core DMA bus, so
            # loads are spread across three issue paths in first-use
            # order: x0 on sync, w1's first block + biases on scalar,
            # the rest on gpsimd (transfers on one queue drain FIFO, so
            # queue placement is explicit priority).
            nc.sync.dma_start(
                out=xb_sb[0][:, :, :], in_=xd[:, 0 : _KC * widths[0]]
            )
            nc.scalar.dma_start(out=w1_sb[:, :, 0:128], in_=w1d[:, :, 0:128])
            nc.scalar.dma_start(out=b_sb[:, :], in_=bd[:, :])
            # Warmup source tile (zeros) for the PE clock ramp.
            nc.vector.memset(warm_sb[:, :], 0.0)
            nc.vector.tensor_copy(x_sb[0][:, :, :], xb_sb[0][:, :, :])
            # w1's later column blocks and x1 arrive bf16-staged (half
            # the DMA-bus bytes in the crunch window). The idle-until-
            # first-relu ACT engine casts w1 up to float32r while the
            # DVE casts x — both in parallel with chunk0's matmuls.
            # w1's second block rides gpsimd as direct fp32 (first in
            # the queue, no cast gating m1-m2); blocks [384:1024] are
            # bf16-staged with DVE-only casts, leaving ACT purely for
            # relus.
            nc.gpsimd.dma_start(out=w1_sb[:, :, 128:384], in_=w1d[:, :, 128:384])
            wsz = [256, 256, 128]
            woff = [384, 640, 896]
            wb_sb = [
                wpool.tile([128, _KC, wsz[j]], bf16, tag=f"wb{j}", name=f"wb{j}")
                for j in range(3)
            ]
            scratch0 = wpool.tile([128, 1], f32r, tag="scratch0")
            for j in range(3):
                if j == 1:
                    # Gate the remaining transfers on x0's arrival: x0 is
                    # the critical load (everything waits on it) and the
                    # queues share one DMA bus, so only w1's fp32 block
                    # and wb0 may compete with it.
                    nc.gpsimd.tensor_copy(scratch0[:, :], xb_sb[0][:, 0, 0:1])
                nc.gpsimd.dma_start(
                    out=wb_sb[j][:, :, :],
                    in_=w1bd[
                        :,
                        _KC * (woff[j] - 384) : _KC * (woff[j] - 384 + wsz[j]),
                    ],
                )
            if len(widths) > 1:
                nc.gpsimd.dma_start(
                    out=xb_sb[1][:, :, :],
                    in_=xd[
                        :,
                        _KC * offsets[1] : _KC * (offsets[1] + widths[1]),
                    ],
                )
            for j in range(3):
                nc.vector.tensor_copy(
                    w1_sb[:, :, woff[j] : woff[j] + wsz[j]], wb_sb[j][:, :, :]
                )
            if len(widths) > 1:
                nc.vector.tensor_copy(x_sb[1][:, :, :], xb_sb[1][:, :, :])
            # x2+ and w2 are prefetched from inside the L1 loop, gated on
            # h-tile reads so their transfers don't compete for DMA-bus
            # with the startup-critical loads.
            scratch = wpool.tile([128, 1], f32r, tag="scratch")
            if len(widths) < 3:
                nc.gpsimd.dma_start(out=w2_sb[:, :, 0:256], in_=w2d[:, :, 0:256])
                nc.gpsimd.dma_start(out=w2_sb[:, :, 256:_D], in_=w2d[:, :, 256:_D])

            # PE warmup: the clock gate holds the PE at reduced rate for
            # the first ~20us; burn the DMA-wait window ramping it.
            for _ in range(9):
                wp = psp.tile([128, 512], fp32, tag="ps", name="warmps")
                nc.tensor.matmul(
                    wp[:, :], warm_sb[:, 0:128], warm_sb[:, 128:640],
                    start=True, stop=True,
                )

            # Phase 1: L1 (h = relu(W1.T x + b1)) for every chunk. All h
            # tiles stay resident in SBUF (~67KB/partition), which pushes
            # W2's load out of the HBM-saturated startup window entirely.
            h_sb: dict = {}
            for ci, nl in enumerate(widths):
                x = x_sb[ci]
                for m in range(mh):
                    ps = psp.tile([128, nl], fp32, tag="ps", name=f"ps1_{ci}_{m}")
                    for k in range(_KC):
                        nc.tensor.matmul(
                            ps[:, :],
                            w1_sb[:, k, m * 128 : (m + 1) * 128],
                            x[:, k, :],
                            start=(k == 0),
                            stop=(k == _KC - 1),
                        )
                    ht = hpool.tile(
                        [128, nl], f32r, tag=f"h_{ci}_{m}", name=f"h_{ci}_{m}"
                    )
                    nc.scalar.activation(
                        ht[:, :], ps[:, :], act.Relu, bias=b_sb[:, m : m + 1]
                    )
                    h_sb[(ci, m)] = ht
                    if m == 0 and ci + 2 < len(widths):
                        # Flow control: block the (in-order) gpsimd queue
                        # until the PE reaches chunk ci, then release the
                        # next deferred transfer.
                        cj = ci + 2
                        nc.gpsimd.tensor_copy(scratch[:, :], h_sb[(ci, 0)][:, 0:1])
                        nc.gpsimd.dma_start(
                            out=xb_sb[cj][:, :, :],
                            in_=xd[
                                :,
                                _KC * offsets[cj] : _KC * (offsets[cj] + widths[cj]),
                            ],
                        )
                        nc.vector.tensor_copy(
                            x_sb[cj][:, :, :], xb_sb[cj][:, :, :]
                        )
                        if cj == len(widths) - 1:
                            # Last deferred x: W2 follows (first needed at
                            # the start of the L2 phase, long after).
                            nc.gpsimd.dma_start(
                                out=w2_sb[:, :, 0:256], in_=w2d[:, :, 0:256]
                            )
                            nc.gpsimd.dma_start(
                                out=w2_sb[:, :, 256:_D], in_=w2d[:, :, 256:_D]
                            )

            # Phase 2: L2 (y = W2.T h + b2), chunks in reverse order so
            # the kernel tail drains the small lead-in chunk.
            for ci in reversed(range(len(widths))):
                nl, n0 = widths[ci], offsets[ci]
                yt = ypool.tile([128, md, nl], bf16, tag="y", name=f"y_{ci}")
                for d in range(md):
                    ps = psp.tile([128, nl], fp32, tag="ps", name=f"ps2_{ci}_{d}")
                    for m in range(mh):
                        nc.tensor.matmul(
                            ps[:, :],
                            w2_sb[:, m, d * 128 : (d + 1) * 128],
                            h_sb[(ci, m)][:, :],
                            start=(m == 0),
                            stop=(m == mh - 1),
                        )
                    nc.vector.tensor_scalar_add(
                        yt[:, d, :], ps[:, :], b_sb[:, mh + d : mh + d + 1]
                    )
                    if ci == 0 and d == md - 2:
                        # Final chunk: flush d0..d2 early so the kernel
                        # tail is a single small d-group store.
                        nc.sync.dma_start(
                            out=od[:, 0 : md - 1, n0 : n0 + nl],
                            in_=yt[:, 0 : md - 1, :],
                        )
                if ci == 0:
                    nc.sync.dma_start(
                        out=od[:, md - 1 :, n0 : n0 + nl], in_=yt[:, md - 1 :, :]
                    )
                else:
                    nc.sync.dma_start(out=od[:, :, n0 : n0 + nl], in_=yt[:, :, :])

    nc.compile()
    return nc


def _get_bass(C: int):
    nc = _kernel_cache.get(C)
    if nc is None:
        nc = _build_bass(C)
        _kernel_cache[C] = nc
    return nc


def _prepare_in_maps(latents, actions, policy_indices, W1, b1, W2, b2):
    """Expert-parallel dispatch: returns (in_maps, C, order, offs, counts)."""
    latents = np.asarray(latents, dtype=np.float32)
    actions = np.asarray(actions, dtype=np.float32)
    pi = np.asarray(policy_indices).astype(np.int64)
    W1 = np.asarray(W1, dtype=np.float32)
    b1 = np.asarray(b1, dtype=np.float32)
    W2 = np.asarray(W2, dtype=np.float32)
    b2 = np.asarray(b2, dtype=np.float32)

    B = latents.shape[0]
    counts = np.bincount(pi, minlength=_P)
    order = np.argsort(pi, kind="stable")
    offs = np.concatenate(([0], np.cumsum(counts)))

    # Exact capacity (rounded to 8 cols): matmul free size has no
    # 128-alignment requirement, and every padded column costs 72
    # PE passes.
    C = max(256, int(math.ceil(counts.max() / 8)) * 8)

    x = np.empty((B, _DA), dtype=np.float32)
    x[:, :_D] = latents
    x[:, _D:] = actions
    x_sorted = x[order]

    mh = _H // 128
    md = _D // 128
    in_maps = []
    for p in range(_P):
        xp = np.zeros((C, _KC * 128), dtype=np.float32)
        xp[: counts[p], :_DA] = x_sorted[offs[p] : offs[p + 1]]
        xr = xp.T.reshape(_KC, 128, C).transpose(1, 0, 2).astype(ml_dtypes.bfloat16)
        widths = _chunks(C)
        noff = [sum(widths[:i]) for i in range(len(widths))]
        xq = np.empty((128, _KC * C), dtype=ml_dtypes.bfloat16)
        for ci, nl in enumerate(widths):
            n0 = noff[ci]
            xq[:, _KC * n0 : _KC * (n0 + nl)] = xr[:, :, n0 : n0 + nl].reshape(
                128, _KC * nl
            )
        w1p = np.zeros((_KC * 128, _H), dtype=np.float32)
        w1p[:_DA] = W1[p]
        w1r = w1p.reshape(_KC, 128, _H).transpose(1, 0, 2)
        w1q = np.ascontiguousarray(w1r[:, :, 0:384])
        w1b3 = w1r[:, :, 384:].astype(ml_dtypes.bfloat16)
        w1qb = np.empty((128, _KC * (_H - 384)), dtype=ml_dtypes.bfloat16)
        for a, b in ((0, 256), (256, 512), (512, 640)):
            w1qb[:, _KC * a : _KC * b] = w1b3[:, :, a:b].reshape(128, _KC * (b - a))
        w2q = np.ascontiguousarray(W2[p].reshape(mh, 128, _D).transpose(1, 0, 2))
        bq = np.empty((128, mh + md), dtype=np.float32)
        bq[:, :mh] = b1[p].reshape(mh, 128).T
        bq[:, mh:] = b2[p].reshape(md, 128).T
        in_maps.append(
            {"xq": xq, "w1q": w1q, "w1qb": w1qb, "w2q": w2q, "bq": bq}
        )
    return in_maps, C, order, offs, counts


def kernel(latents, actions, policy_indices, W1, b1, W2, b2):
    from concourse.bass_utils import run_bass_kernel_spmd

    in_maps, C, order, offs, counts = _prepare_in_maps(
        latents, actions, policy_indices, W1, b1, W2, b2
    )
    nc = _get_bass(C)
    results = run_bass_kernel_spmd(nc, in_maps, list(range(_N_CORES))).results

    B = np.asarray(latents).shape[0]
    out = np.empty((B, _D), dtype=np.float32)
    for p in range(_P):
        oq = np.asarray(results[p]["oq"])  # [128, 4, C] bf16
        yT = oq.transpose(1, 0, 2).reshape(_D, C)
        out[order[offs[p] : offs[p + 1]]] = yT[:, : counts[p]].T.astype(np.float32)
    return out

